# revision 16
# baseline (speedup 1.0000x reference)
"""Trainium2 Bass kernel for nn_MinimalLoss (YOLO-style detection loss).

Sharding strategy (data-parallel over 8 NeuronCores, 4 batches each):
  * predictions are sharded along B (each core gets its contiguous
    [4*25600, 85] slab, used only for the per-target indirect row gather);
  * the conf channel (column 4) is additionally staged as its own
    contiguous per-core [128, 800] tensor -- a channel-axis shard of
    predictions.  This turns the dominant data access (sum over all cells
    of ln(1-sigmoid(conf))) from a 4-byte-strided DMA (descriptor-rate
    bound, ~78us of SDMA busy) into one 400KB contiguous DMA (~1us).
  * targets are sharded along B and staged slot-packed/field-major as
    [100, 10] so every per-field access on device is a contiguous slice.
  * each core returns raw partial sums ([128, 11]); the final all-reduce
    of the 5 scalar loss terms happens on host in fp64.

Device math (all on-chip):
  * -ln(1-sigmoid(x)) = softplus(x): ONE activation pass with accum_out
    over the conf shard gives per-partition partial sums.
  * conf correction at an object cell: ln(1-s)-ln(s) = -x exactly, so the
    correction is just the gathered conf logit (first-occurrence weighted).
  * bce_cls per target = (sum_c softplus(x_c) - x_cls)/C exactly.
  * pred_xy = sigmoid(rows[:, 0:2]), pred_wh = exp(rows[:, 2:4]) via ACT.
  * duplicate-cell targets deduplicated with transpose/is_equal
    first-occurrence matrix per slot (2 whole batches per slot, so
    duplicates never cross slots).
"""
import numpy as np

import concourse.bass as bass
import concourse.mybir as mybir
import concourse.tile as tile
from concourse.bass import IndirectOffsetOnAxis
from concourse.masks import make_identity

F32 = mybir.dt.float32
I32 = mybir.dt.int32
AF = mybir.ActivationFunctionType
ALU = mybir.AluOpType
AX = mybir.AxisListType

B, HWC, C, T = 32, 25600, 80, 50          # full problem
H = W = 160
NCORES = 8
BL = B // NCORES                          # 4 batches per core
ROWS = BL * HWC                           # 102400 prediction rows per core
NT = BL * T                               # 200 targets per core
P = 100                                   # targets per slot (partition dim)
NS = 2                                    # slots (each = 2 whole batches)
CONF_P, CONF_F = 128, ROWS // 128         # conf shard layout [128, 800]
MAGIC = float(np.float32(2 ** 23))


def _split_multi_waits(nc):
    """Walrus codegen accepts at most ONE sync wait per instruction; hoist
    extras onto standalone EventSemaphore (wait) ops on the same engine."""
    n = 0
    for func in nc.m.functions:
        for block in func.blocks:
            out = []
            for inst in block.instructions:
                si = inst.sync_info
                if si is not None and si.on_wait and len(si.on_wait) > 1:
                    waits = list(si.on_wait)
                    for w in waits[:-1]:
                        n += 1
                        nop = mybir.InstEventSemaphore(
                            name=f"{inst.name}_sw{n}", engine=inst.engine,
                            ins=[], outs=[])
                        nop.sync_info = mybir.SyncInfo(on_wait=[w], on_update=[])
                        out.append(nop)
                    inst.sync_info = mybir.SyncInfo(on_wait=[waits[-1]],
                                                    on_update=list(si.on_update))
                out.append(inst)
            if n:
                block.instructions[:] = out
    return n


def build_nc(split=True):
    nc = bass.Bass("TRN2", target_bir_lowering=False, debug=False)
    pred_d = nc.dram_tensor("predictions", [ROWS, 85], F32, kind="ExternalInput")
    conf_d = nc.dram_tensor("conf", [CONF_P, CONF_F], F32, kind="ExternalInput")
    tgt_d = nc.dram_tensor("targets", [P, NS * 5], F32, kind="ExternalInput")
    out_d = nc.dram_tensor("out", [128, 11], F32, kind="ExternalOutput")

    with tile.TileContext(nc) as tc:
        with tc.tile_pool(name="pp", bufs=1) as pp, \
             tc.tile_pool(name="ps", bufs=1, space="PSUM") as ps:

            # ---- ACT table preload: a dummy Exp forces the exp/ln PWP set
            # to load during the prologue instead of blocking the first real
            # activation (all ACT funcs below are exp/ln = one table set).
            dummy = pp.tile([1, 1], F32)
            nc.vector.memset(dummy[:], 0.0)
            nc.scalar.activation(out=dummy[:], in_=dummy[:], func=AF.Exp)

            # ---- input DMAs, issued first on separate queues (targets head
            # the critical path, so they go out first)
            tt = pp.tile([P, NS * 5], F32)
            nc.sync.dma_start(out=tt[:], in_=tgt_d.ap())
            conf_t = pp.tile([CONF_P, CONF_F], F32)
            nc.scalar.dma_start(out=conf_t[:], in_=conf_d.ap())
            # tt cols (slot-major xy / wh, then cls):
            #   0:4  = {x0,y0,x1,y1}, 4:8 = {w0,h0,w1,h1}, 8:10 = {cls0,cls1}

            # ---- constants (DVE/gpsimd, overlap with the DMAs)
            ident_g = pp.tile([128, 128], F32)
            make_identity(nc, ident_g[:])
            ident = pp.tile([128, 128], F32)
            nc.vector.tensor_copy(out=ident[:], in_=ident_g[:])

            iotac = pp.tile([P, C], I32)
            nc.gpsimd.iota(iotac[:], pattern=[[1, C]], base=0, channel_multiplier=0)
            iotaf = pp.tile([P, C], F32)
            nc.vector.tensor_copy(out=iotaf[:], in_=iotac[:])

            iotap = pp.tile([P, 1], I32)
            nc.gpsimd.iota(iotap[:], pattern=[[1, 1]], base=0, channel_multiplier=1)
            pf = pp.tile([P, 1], F32)
            nc.vector.tensor_copy(out=pf[:], in_=iotap[:])

            iotar = pp.tile([P, P], I32)
            nc.gpsimd.iota(iotar[:], pattern=[[1, P]], base=0, channel_multiplier=0)
            iotarf = pp.tile([P, P], F32)
            nc.vector.tensor_copy(out=iotarf[:], in_=iotar[:])
            tri = pp.tile([P, P], F32)  # tri[p, f] = 1.0 iff f < p
            nc.vector.tensor_tensor(out=tri[:], in0=pf[:].to_broadcast([P, P]),
                                    in1=iotarf[:], op=ALU.is_gt)

            # negk[p, j] = -(1 + p + 100*j): unique negative dedup keys
            negi = pp.tile([P, NS], I32)
            nc.gpsimd.iota(negi[:], pattern=[[P, NS]], base=1, channel_multiplier=1)
            negk = pp.tile([P, NS], F32)
            nc.vector.tensor_copy(out=negk[:], in_=negi[:])
            nc.vector.tensor_scalar_mul(negk[:], negk[:], -1.0)

            # boff[p, j] = HWC * (2j + (p >= 50)): batch row offset
            jci = pp.tile([P, NS], I32)
            nc.gpsimd.iota(jci[:], pattern=[[1, NS]], base=0, channel_multiplier=0)
            boff = pp.tile([P, NS], F32)
            nc.vector.tensor_copy(out=boff[:], in_=jci[:])
            nc.vector.tensor_scalar_mul(boff[:], boff[:], float(2 * HWC))
            par = pp.tile([P, 1], F32)
            nc.vector.tensor_scalar(out=par[:], in0=pf[:], scalar1=float(T),
                                    scalar2=float(HWC), op0=ALU.is_ge, op1=ALU.mult)
            nc.vector.tensor_tensor(out=boff[:], in0=boff[:],
                                    in1=par[:].to_broadcast([P, NS]), op=ALU.add)

            # ---- conf term: sum softplus(conf) = sum ln(exp(conf) + 1).
            # Only exp/ln tables are used kernel-wide (one PWP table set; no
            # native softplus table on TRN2); the +1 rides the Ln bias input.
            csp = pp.tile([CONF_P, 1], F32)
            e_conf = pp.tile([CONF_P, CONF_F], F32)
            nc.scalar.activation(out=e_conf[:], in_=conf_t[:], func=AF.Exp)
            spdump = pp.tile([CONF_P, CONF_F], F32)
            nc.scalar.activation(out=spdump[:], in_=e_conf[:], func=AF.Ln,
                                 bias=1.0, accum_out=csp[:])

            # ---- per-target index chain (slot-major [P, 4] = {x0,y0,x1,y1});
            # emission order = DVE execution order, so the idx chain comes
            # first and everything gather-independent fills the gather gap.
            xw8 = pp.tile([P, 8], F32)   # {x,y}*W slot-major | {w,h}*W slot-major
            nc.vector.tensor_scalar_mul(xw8[:], tt[:, 0:8], float(W))
            xyW = xw8[:, 0:4]
            twh = xw8[:, 4:8]

            # floor via round-to-nearest magic + fixup
            g_r = pp.tile([P, 4], F32)
            nc.vector.tensor_scalar_add(g_r[:], xyW, MAGIC)
            nc.vector.tensor_scalar_add(g_r[:], g_r[:], -MAGIC)
            g_adj = pp.tile([P, 4], F32)
            nc.vector.tensor_tensor(out=g_adj[:], in0=g_r[:], in1=xyW, op=ALU.is_gt)
            gxy = pp.tile([P, 4], F32)
            nc.vector.tensor_tensor(out=gxy[:], in0=g_r[:], in1=g_adj[:], op=ALU.subtract)

            # cell = gy*W + gx (strided {y0,y1} / {x0,x1} views), row index
            gcl = pp.tile([P, 4], F32)
            nc.vector.tensor_scalar(out=gcl[:], in0=gxy[:], scalar1=0.0,
                                    scalar2=float(W - 1), op0=ALU.max, op1=ALU.min)
            gv = gcl[:].rearrange("p (j c) -> p c j", c=2)     # [P, coord, slot]
            cell = pp.tile([P, NS], F32)
            cv = cell[:].rearrange("p (o j) -> p o j", o=1)    # [P, 1, slot]
            nc.vector.tensor_scalar(out=cv, in0=gv[:, 1:2, :], scalar1=float(W),
                                    scalar2=None, op0=ALU.mult)
            nc.vector.tensor_tensor(out=cv, in0=cv, in1=gv[:, 0:1, :], op=ALU.add)
            rowf = pp.tile([P, NS], F32)
            nc.vector.tensor_tensor(out=rowf[:], in0=cell[:], in1=boff[:], op=ALU.add)
            idx = pp.tile([P, NS], I32)
            nc.vector.tensor_copy(out=idx[:], in_=rowf[:])

            # ---- gather prediction rows: single indirect DMA, 2 offsets per
            # partition -> rows2[p] = [pred[idx[p,0]], pred[idx[p,1]]]
            rows2 = pp.tile([P, NS * 85], F32)
            nc.gpsimd.indirect_dma_start(
                out=rows2[:].rearrange("p (j c) -> p j c", c=85), out_offset=None,
                in_=pred_d.ap()[:, :],
                in_offset=IndirectOffsetOnAxis(ap=idx[:, :], axis=0))
            rows = [rows2[:, 85 * j:85 * j + 85] for j in range(NS)]

            # ---- gather-independent work (fills the gather latency):
            # validity, dedup keys, regression targets
            v4 = pp.tile([P, 4], F32)
            t4 = pp.tile([P, 4], F32)
            nc.vector.tensor_scalar(out=v4[:], in0=gxy[:], scalar1=0.0, scalar2=None,
                                    op0=ALU.is_ge)
            nc.vector.tensor_scalar(out=t4[:], in0=gxy[:], scalar1=float(W), scalar2=None,
                                    op0=ALU.is_lt)
            nc.vector.tensor_tensor(out=v4[:], in0=v4[:], in1=t4[:], op=ALU.mult)
            vf = pp.tile([P, NS], F32)
            nc.vector.tensor_tensor(out=vf[:, 0:1], in0=v4[:, 0:1], in1=v4[:, 1:2],
                                    op=ALU.mult)
            nc.vector.tensor_tensor(out=vf[:, 1:2], in0=v4[:, 2:3], in1=v4[:, 3:4],
                                    op=ALU.mult)

            # dedup key: valid -> rowf ; invalid -> unique negative
            key = pp.tile([P, NS], F32)
            nc.vector.tensor_tensor(out=key[:], in0=rowf[:], in1=negk[:], op=ALU.subtract)
            nc.vector.tensor_tensor(out=key[:], in0=key[:], in1=vf[:], op=ALU.mult)
            nc.vector.tensor_tensor(out=key[:], in0=key[:], in1=negk[:], op=ALU.add)

            # txy and 1-txy (dxy = sigmoid - txy = (1-txy) - 1/(1+exp(x)))
            txy = pp.tile([P, 4], F32)
            nc.vector.tensor_tensor(out=txy[:], in0=xyW, in1=gxy[:], op=ALU.subtract)
            onemt = pp.tile([P, 4], F32)
            nc.vector.tensor_scalar(out=onemt[:], in0=txy[:], scalar1=-1.0, scalar2=1.0,
                                    op0=ALU.mult, op1=ALU.add)

            # ---- per-slot ACT passes (exp/ln only):
            #   sum_c softplus(cls logits) via ln(1+exp(x)) with accum_out;
            #   e4 = exp(xywh logits): wh uses it directly, sigmoid = 1-1/(1+e).
            # ---- dedup (gather-independent): first-occurrence weight per slot
            dup = pp.tile([P, NS], F32)
            for j in range(NS):
                keyT_ps = ps.tile([P, P], F32, space="PSUM", tag=f"keyT{j}")
                nc.tensor.transpose(out=keyT_ps[:], in_=key[:, j:j + 1].to_broadcast([P, P]),
                                    identity=ident[:P, :P])
                keyT_sb = pp.tile([P, P], F32)
                nc.vector.tensor_copy(out=keyT_sb[:], in_=keyT_ps[:])
                nc.vector.tensor_tensor(out=keyT_sb[:], in0=key[:, j:j + 1].to_broadcast([P, P]),
                                        in1=keyT_sb[:], op=ALU.is_equal)
                nc.vector.tensor_tensor(out=keyT_sb[:], in0=keyT_sb[:], in1=tri[:], op=ALU.mult)
                nc.vector.reduce_max(out=dup[:, j:j + 1], in_=keyT_sb[:], axis=AX.X)
            wfo = pp.tile([P, NS], F32)
            nc.vector.tensor_scalar(out=wfo[:], in0=dup[:], scalar1=-1.0, scalar2=1.0,
                                    op0=ALU.mult, op1=ALU.add)
            nc.vector.tensor_tensor(out=wfo[:], in0=wfo[:], in1=vf[:], op=ALU.mult)

            # ---- post-gather: ACT does softplus-sums (cls) and exp(xywh);
            # DVE finishes sigmoid via reciprocal and the onehot dot.
            spc = pp.tile([P, NS], F32)
            rxy = pp.tile([P, 4], F32)   # 1/(1+exp(x)) per slot
            e4s = []
            for j in range(NS):
                e80 = pp.tile([P, C], F32, name=f"e80_{j}")
                nc.scalar.activation(out=e80[:], in_=rows[j][:, 5:85], func=AF.Exp)
                spdump2 = pp.tile([P, C], F32, name=f"spdump2_{j}")
                nc.scalar.activation(out=spdump2[:], in_=e80[:], func=AF.Ln,
                                     bias=1.0, accum_out=spc[:, j:j + 1])
                e4 = pp.tile([P, 4], F32, name=f"e4_{j}")
                nc.scalar.activation(out=e4[:], in_=rows[j][:, 0:4], func=AF.Exp)
                e4s.append(e4)
                nc.vector.tensor_scalar_add(e4[:, 0:2], e4[:, 0:2], 1.0)
                nc.vector.reciprocal(out=rxy[:, 2 * j:2 * j + 2], in_=e4[:, 0:2])

            # dxy = sigmoid - txy = (1-txy) - 1/(1+exp(x)); dwh = exp(x) - twh
            dxy = pp.tile([P, 4], F32)
            nc.vector.tensor_tensor(out=dxy[:], in0=onemt[:], in1=rxy[:], op=ALU.subtract)
            nc.vector.tensor_tensor(out=dxy[:], in0=dxy[:], in1=dxy[:], op=ALU.mult)
            dwh = pp.tile([P, 4], F32)
            for j in range(NS):
                nc.vector.tensor_tensor(out=dwh[:, 2 * j:2 * j + 2], in0=e4s[j][:, 2:4],
                                        in1=twh[:, 2 * j:2 * j + 2], op=ALU.subtract)
            nc.vector.tensor_tensor(out=dwh[:], in0=dwh[:], in1=dwh[:], op=ALU.mult)
            pxy = pp.tile([P, NS], F32)
            pwh = pp.tile([P, NS], F32)
            for j in range(NS):
                nc.vector.tensor_tensor(out=pxy[:, j:j + 1], in0=dxy[:, 2 * j:2 * j + 1],
                                        in1=dxy[:, 2 * j + 1:2 * j + 2], op=ALU.add)
                nc.vector.tensor_tensor(out=pwh[:, j:j + 1], in0=dwh[:, 2 * j:2 * j + 1],
                                        in1=dwh[:, 2 * j + 1:2 * j + 2], op=ALU.add)

            # x_cls extraction: onehot dot product per slot
            xcls = pp.tile([P, NS], F32)
            for j in range(NS):
                oh = pp.tile([P, C], F32)
                nc.vector.tensor_tensor(out=oh[:], in0=iotaf[:],
                                        in1=tt[:, 8 + j:9 + j].to_broadcast([P, C]),
                                        op=ALU.is_equal)
                nc.vector.tensor_tensor(out=oh[:], in0=oh[:], in1=rows[j][:, 5:85],
                                        op=ALU.mult)
                nc.vector.reduce_sum(out=xcls[:, j:j + 1], in_=oh[:], axis=AX.X)
            pcls = pp.tile([P, NS], F32)   # = C * per_cls
            nc.vector.tensor_tensor(out=pcls[:], in0=spc[:], in1=xcls[:], op=ALU.subtract)

            # ---- raw per-target stats -> out rows 0:100, cols 0:10
            # cols: [vf*pxy(2), vf*pwh(2), vf*pcls(2), vf(2), w*x4(2)]
            out_t = pp.tile([128, 11], F32)
            nc.vector.memset(out_t[:], 0.0)
            st = out_t[:P, :]
            nc.vector.tensor_tensor(out=st[:, 0:2], in0=pxy[:], in1=vf[:], op=ALU.mult)
            nc.vector.tensor_tensor(out=st[:, 2:4], in0=pwh[:], in1=vf[:], op=ALU.mult)
            nc.vector.tensor_tensor(out=st[:, 4:6], in0=pcls[:], in1=vf[:], op=ALU.mult)
            nc.vector.tensor_copy(out=st[:, 6:8], in_=vf[:])
            for j in range(NS):
                nc.vector.tensor_tensor(out=st[:, 8 + j:9 + j], in0=rows[j][:, 4:5],
                                        in1=wfo[:, j:j + 1], op=ALU.mult)
            nc.vector.tensor_copy(out=out_t[:, 10:11], in_=csp[:])
            nc.sync.dma_start(out=out_d.ap(), in_=out_t[:])
    if split:
        _split_multi_waits(nc)
    return nc


_NC_CACHE = None


def _get_nc():
    global _NC_CACHE
    if _NC_CACHE is None:
        _NC_CACHE = build_nc()
    return _NC_CACHE


def make_in_maps(predictions, targets):
    preds = np.ascontiguousarray(np.asarray(predictions, dtype=np.float32)).reshape(
        NCORES, ROWS, 85)
    # channel-axis shard: conf column staged contiguously per core
    conf = np.ascontiguousarray(preds[:, :, 4]).reshape(NCORES, CONF_P, CONF_F)
    # targets: [NCORES, 4, 50, 5] -> slot-packed [100, (xy slot-major, wh, cls)]
    tg = np.ascontiguousarray(np.asarray(targets, dtype=np.float32)).reshape(
        NCORES, BL, T, 5)
    # z[c, parity, t, j, f] = tg[c, 2j+parity, t, f]
    z = tg.reshape(NCORES, 2, 2, T, 5).transpose(0, 2, 3, 1, 4)  # [c,parity,t,j,f]
    cls_ = z[..., 0]                                   # [c,parity,t,j]
    xy = z[..., 1:3]                                   # [c,parity,t,j,2]
    wh = z[..., 3:5]
    tt = np.concatenate([
        xy.reshape(NCORES, P, 4),                      # {x0,y0,x1,y1}
        wh.reshape(NCORES, P, 4),                      # {w0,h0,w1,h1}
        cls_.reshape(NCORES, P, 2),                    # {cls0,cls1}
    ], axis=2)
    tt = np.ascontiguousarray(tt)
    return [{"predictions": preds[c], "conf": conf[c], "targets": tt[c]}
            for c in range(NCORES)]


def combine_partials(parts):
    """parts: list of 8 arrays [128,11] -> (total, loss_xy, loss_wh, loss_conf, loss_cls)"""
    sxy = swh = scls = nt = corr = spsum = 0.0
    for p in parts:
        a = np.asarray(p, dtype=np.float64)
        st = a[:P, 0:10]
        sxy += st[:, 0:2].sum()
        swh += st[:, 2:4].sum()
        scls += st[:, 4:6].sum()
        nt += st[:, 6:8].sum()
        corr += st[:, 8:10].sum()
        spsum += a[:, 10].sum()
    denom = max(nt, 1.0)
    loss_xy = np.float32(0.5 * sxy / denom)
    loss_wh = np.float32(0.5 * swh / denom)
    loss_cls = np.float32(scls / C / denom)
    loss_conf = np.float32((spsum - corr) / float(B * HWC))
    total = np.float32(5.0 * float(loss_xy) + 5.0 * float(loss_wh)
                       + float(loss_conf) + float(loss_cls))
    return total, loss_xy, loss_wh, loss_conf, loss_cls


def kernel(predictions, targets, H=None, W=None):
    from concourse.bass_utils import run_bass_kernel_spmd

    nc = _get_nc()
    in_maps = make_in_maps(predictions, targets)
    res = run_bass_kernel_spmd(nc, in_maps, core_ids=list(range(NCORES)))
    parts = [res.results[c]["out"] for c in range(NCORES)]
    return combine_partials(parts)


# revision 17
# speedup vs baseline: 1.0739x; 1.0739x over previous
"""Trainium2 Bass kernel for nn_MinimalLoss (YOLO-style detection loss).

Sharding strategy (data-parallel over 8 NeuronCores, 4 batches each):
  * predictions are sharded along B (each core gets its contiguous
    [4*25600, 85] slab, used only for the per-target indirect row gather);
  * the conf channel (column 4) is additionally staged as its own
    contiguous per-core [128, 800] tensor -- a channel-axis shard of
    predictions.  This turns the dominant data access (sum over all cells
    of ln(1-sigmoid(conf))) from a 4-byte-strided DMA (descriptor-rate
    bound, ~78us of SDMA busy) into one 400KB contiguous DMA (~1us).
  * targets are sharded along B and staged slot-packed/field-major as
    [100, 10] so every per-field access on device is a contiguous slice.
  * each core returns raw partial sums ([128, 11]); the final all-reduce
    of the 5 scalar loss terms happens on host in fp64.

Device math (all on-chip):
  * -ln(1-sigmoid(x)) = softplus(x): ONE activation pass with accum_out
    over the conf shard gives per-partition partial sums.
  * conf correction at an object cell: ln(1-s)-ln(s) = -x exactly, so the
    correction is just the gathered conf logit (first-occurrence weighted).
  * bce_cls per target = (sum_c softplus(x_c) - x_cls)/C exactly.
  * pred_xy = sigmoid(rows[:, 0:2]), pred_wh = exp(rows[:, 2:4]) via ACT.
  * duplicate-cell targets deduplicated with transpose/is_equal
    first-occurrence matrix per slot (2 whole batches per slot, so
    duplicates never cross slots).
"""
import numpy as np

import concourse.bass as bass
import concourse.mybir as mybir
import concourse.tile as tile
from concourse.bass import IndirectOffsetOnAxis
from concourse.masks import make_identity

F32 = mybir.dt.float32
I32 = mybir.dt.int32
AF = mybir.ActivationFunctionType
ALU = mybir.AluOpType
AX = mybir.AxisListType

B, HWC, C, T = 32, 25600, 80, 50          # full problem
H = W = 160
NCORES = 8
BL = B // NCORES                          # 4 batches per core
ROWS = BL * HWC                           # 102400 prediction rows per core
NT = BL * T                               # 200 targets per core
P = 100                                   # targets per slot (partition dim)
NS = 2                                    # slots (each = 2 whole batches)
CONF_P, CONF_F = 128, ROWS // 128         # conf shard layout [128, 800]
MAGIC = float(np.float32(2 ** 23))


def _split_multi_waits(nc):
    """Walrus codegen accepts at most ONE sync wait per instruction; hoist
    extras onto standalone EventSemaphore (wait) ops on the same engine."""
    n = 0
    for func in nc.m.functions:
        for block in func.blocks:
            out = []
            for inst in block.instructions:
                si = inst.sync_info
                if si is not None and si.on_wait and len(si.on_wait) > 1:
                    waits = list(si.on_wait)
                    for w in waits[:-1]:
                        n += 1
                        nop = mybir.InstEventSemaphore(
                            name=f"{inst.name}_sw{n}", engine=inst.engine,
                            ins=[], outs=[])
                        nop.sync_info = mybir.SyncInfo(on_wait=[w], on_update=[])
                        out.append(nop)
                    inst.sync_info = mybir.SyncInfo(on_wait=[waits[-1]],
                                                    on_update=list(si.on_update))
                out.append(inst)
            if n:
                block.instructions[:] = out
    return n


def build_nc(split=True):
    nc = bass.Bass("TRN2", target_bir_lowering=False, debug=False)
    pred_d = nc.dram_tensor("predictions", [ROWS, 85], F32, kind="ExternalInput")
    conf_d = nc.dram_tensor("conf", [CONF_P, CONF_F], F32, kind="ExternalInput")
    tgt_d = nc.dram_tensor("targets", [P, NS * 5], F32, kind="ExternalInput")
    out_d = nc.dram_tensor("out", [128, 11], F32, kind="ExternalOutput")

    with tile.TileContext(nc) as tc:
        with tc.tile_pool(name="pp", bufs=1) as pp, \
             tc.tile_pool(name="ps", bufs=1, space="PSUM") as ps:

            # ---- ACT table preload: a dummy Exp forces the exp/ln PWP set
            # to load during the prologue instead of blocking the first real
            # activation (all ACT funcs below are exp/ln = one table set).
            dummy = pp.tile([1, 1], F32)
            nc.vector.memset(dummy[:], 0.0)
            nc.scalar.activation(out=dummy[:], in_=dummy[:], func=AF.Exp)

            # ---- input DMAs, issued first on separate queues (targets head
            # the critical path, so they go out first)
            tt = pp.tile([P, NS * 5], F32)
            nc.sync.dma_start(out=tt[:], in_=tgt_d.ap())
            conf_t = pp.tile([CONF_P, CONF_F], F32)
            nc.scalar.dma_start(out=conf_t[:], in_=conf_d.ap())
            # tt cols (slot-major xy / wh, then cls):
            #   0:4  = {x0,y0,x1,y1}, 4:8 = {w0,h0,w1,h1}, 8:10 = {cls0,cls1}

            # ---- constants (DVE/gpsimd, overlap with the DMAs)
            ident_g = pp.tile([128, 128], F32)
            make_identity(nc, ident_g[:])
            ident = pp.tile([128, 128], F32)
            nc.vector.tensor_copy(out=ident[:], in_=ident_g[:])

            iotac = pp.tile([P, C], I32)
            nc.gpsimd.iota(iotac[:], pattern=[[1, C]], base=0, channel_multiplier=0)
            iotaf = pp.tile([P, C], F32)
            nc.vector.tensor_copy(out=iotaf[:], in_=iotac[:])

            iotap = pp.tile([P, 1], I32)
            nc.gpsimd.iota(iotap[:], pattern=[[1, 1]], base=0, channel_multiplier=1)
            pf = pp.tile([P, 1], F32)
            nc.vector.tensor_copy(out=pf[:], in_=iotap[:])

            iotar = pp.tile([P, P], I32)
            nc.gpsimd.iota(iotar[:], pattern=[[1, P]], base=0, channel_multiplier=0)
            iotarf = pp.tile([P, P], F32)
            nc.vector.tensor_copy(out=iotarf[:], in_=iotar[:])
            tri = pp.tile([P, P], F32)  # tri[p, f] = 1.0 iff f < p
            nc.vector.tensor_tensor(out=tri[:], in0=pf[:].to_broadcast([P, P]),
                                    in1=iotarf[:], op=ALU.is_gt)

            # negk[p, j] = -(1 + p + 100*j): unique negative dedup keys
            negi = pp.tile([P, NS], I32)
            nc.gpsimd.iota(negi[:], pattern=[[P, NS]], base=1, channel_multiplier=1)
            negk = pp.tile([P, NS], F32)
            nc.vector.tensor_copy(out=negk[:], in_=negi[:])
            nc.vector.tensor_scalar_mul(negk[:], negk[:], -1.0)

            # boff[p, j] = HWC * (2j + (p >= 50)): batch row offset
            jci = pp.tile([P, NS], I32)
            nc.gpsimd.iota(jci[:], pattern=[[1, NS]], base=0, channel_multiplier=0)
            boff = pp.tile([P, NS], F32)
            nc.vector.tensor_copy(out=boff[:], in_=jci[:])
            nc.vector.tensor_scalar_mul(boff[:], boff[:], float(2 * HWC))
            par = pp.tile([P, 1], F32)
            nc.vector.tensor_scalar(out=par[:], in0=pf[:], scalar1=float(T),
                                    scalar2=float(HWC), op0=ALU.is_ge, op1=ALU.mult)
            nc.vector.tensor_tensor(out=boff[:], in0=boff[:],
                                    in1=par[:].to_broadcast([P, NS]), op=ALU.add)

            # ---- conf term: sum softplus(conf) = sum ln(exp(conf) + 1).
            # Only exp/ln tables are used kernel-wide (one PWP table set; no
            # native softplus table on TRN2); the +1 rides the Ln bias input.
            csp = pp.tile([CONF_P, 1], F32)
            e_conf = pp.tile([CONF_P, CONF_F], F32)
            nc.scalar.activation(out=e_conf[:], in_=conf_t[:], func=AF.Exp)
            spdump = pp.tile([CONF_P, CONF_F], F32)
            nc.scalar.activation(out=spdump[:], in_=e_conf[:], func=AF.Ln,
                                 bias=1.0, accum_out=csp[:])

            # ---- per-target index chain (slot-major [P, 4] = {x0,y0,x1,y1});
            # emission order = DVE execution order, so the idx chain comes
            # first and everything gather-independent fills the gather gap.
            xw8 = pp.tile([P, 8], F32)   # {x,y}*W slot-major | {w,h}*W slot-major
            nc.vector.tensor_scalar_mul(xw8[:], tt[:, 0:8], float(W))
            xyW = xw8[:, 0:4]
            twh = xw8[:, 4:8]

            # floor via round-to-nearest magic + fixup
            g_r = pp.tile([P, 4], F32)
            nc.vector.tensor_scalar_add(g_r[:], xyW, MAGIC)
            nc.vector.tensor_scalar_add(g_r[:], g_r[:], -MAGIC)
            g_adj = pp.tile([P, 4], F32)
            nc.vector.tensor_tensor(out=g_adj[:], in0=g_r[:], in1=xyW, op=ALU.is_gt)
            gxy = pp.tile([P, 4], F32)
            nc.vector.tensor_tensor(out=gxy[:], in0=g_r[:], in1=g_adj[:], op=ALU.subtract)

            # cell = gy*W + gx (strided {y0,y1} / {x0,x1} views), row index
            gcl = pp.tile([P, 4], F32)
            nc.vector.tensor_scalar(out=gcl[:], in0=gxy[:], scalar1=0.0,
                                    scalar2=float(W - 1), op0=ALU.max, op1=ALU.min)
            gv = gcl[:].rearrange("p (j c) -> p c j", c=2)     # [P, coord, slot]
            cell = pp.tile([P, NS], F32)
            cv = cell[:].rearrange("p (o j) -> p o j", o=1)    # [P, 1, slot]
            nc.vector.tensor_scalar(out=cv, in0=gv[:, 1:2, :], scalar1=float(W),
                                    scalar2=None, op0=ALU.mult)
            nc.vector.tensor_tensor(out=cv, in0=cv, in1=gv[:, 0:1, :], op=ALU.add)
            rowf = pp.tile([P, NS], F32)
            nc.vector.tensor_tensor(out=rowf[:], in0=cell[:], in1=boff[:], op=ALU.add)
            idx = pp.tile([P, NS], I32)
            nc.vector.tensor_copy(out=idx[:], in_=rowf[:])

            # ---- gather prediction rows (one indirect DMA per slot, into
            # adjacent halves of one tile so strided both-slot views work)
            rows2 = pp.tile([P, NS * 85], F32)
            for j in range(NS):
                nc.gpsimd.indirect_dma_start(
                    out=rows2[:, 85 * j:85 * j + 85], out_offset=None,
                    in_=pred_d.ap()[:, :],
                    in_offset=IndirectOffsetOnAxis(ap=idx[:, j:j + 1], axis=0))
            rows = [rows2[:, 85 * j:85 * j + 85] for j in range(NS)]

            # ---- gather-independent work (fills the gather latency):
            # validity, dedup keys, regression targets
            v4 = pp.tile([P, 4], F32)
            t4 = pp.tile([P, 4], F32)
            nc.vector.tensor_scalar(out=v4[:], in0=gxy[:], scalar1=0.0, scalar2=None,
                                    op0=ALU.is_ge)
            nc.vector.tensor_scalar(out=t4[:], in0=gxy[:], scalar1=float(W), scalar2=None,
                                    op0=ALU.is_lt)
            nc.vector.tensor_tensor(out=v4[:], in0=v4[:], in1=t4[:], op=ALU.mult)
            vf = pp.tile([P, NS], F32)
            nc.vector.tensor_tensor(out=vf[:, 0:1], in0=v4[:, 0:1], in1=v4[:, 1:2],
                                    op=ALU.mult)
            nc.vector.tensor_tensor(out=vf[:, 1:2], in0=v4[:, 2:3], in1=v4[:, 3:4],
                                    op=ALU.mult)

            # dedup key: valid -> rowf ; invalid -> unique negative
            key = pp.tile([P, NS], F32)
            nc.vector.tensor_tensor(out=key[:], in0=rowf[:], in1=negk[:], op=ALU.subtract)
            nc.vector.tensor_tensor(out=key[:], in0=key[:], in1=vf[:], op=ALU.mult)
            nc.vector.tensor_tensor(out=key[:], in0=key[:], in1=negk[:], op=ALU.add)

            # txy and 1-txy (dxy = sigmoid - txy = (1-txy) - 1/(1+exp(x)))
            txy = pp.tile([P, 4], F32)
            nc.vector.tensor_tensor(out=txy[:], in0=xyW, in1=gxy[:], op=ALU.subtract)
            onemt = pp.tile([P, 4], F32)
            nc.vector.tensor_scalar(out=onemt[:], in0=txy[:], scalar1=-1.0, scalar2=1.0,
                                    op0=ALU.mult, op1=ALU.add)

            # ---- per-slot ACT passes (exp/ln only):
            #   sum_c softplus(cls logits) via ln(1+exp(x)) with accum_out;
            #   e4 = exp(xywh logits): wh uses it directly, sigmoid = 1-1/(1+e).
            # ---- dedup (gather-independent): first-occurrence weight per slot
            dup = pp.tile([P, NS], F32)
            for j in range(NS):
                keyT_ps = ps.tile([P, P], F32, space="PSUM", tag=f"keyT{j}")
                nc.tensor.transpose(out=keyT_ps[:], in_=key[:, j:j + 1].to_broadcast([P, P]),
                                    identity=ident[:P, :P])
                keyT_sb = pp.tile([P, P], F32)
                nc.vector.tensor_copy(out=keyT_sb[:], in_=keyT_ps[:])
                nc.vector.tensor_tensor(out=keyT_sb[:], in0=key[:, j:j + 1].to_broadcast([P, P]),
                                        in1=keyT_sb[:], op=ALU.is_equal)
                nc.vector.tensor_tensor(out=keyT_sb[:], in0=keyT_sb[:], in1=tri[:], op=ALU.mult)
                nc.vector.reduce_max(out=dup[:, j:j + 1], in_=keyT_sb[:], axis=AX.X)
            wfo = pp.tile([P, NS], F32)
            nc.vector.tensor_scalar(out=wfo[:], in0=dup[:], scalar1=-1.0, scalar2=1.0,
                                    op0=ALU.mult, op1=ALU.add)
            nc.vector.tensor_tensor(out=wfo[:], in0=wfo[:], in1=vf[:], op=ALU.mult)

            # ---- post-gather: ACT does softplus-sums (cls) and exp(xywh);
            # DVE finishes sigmoid via reciprocal and the onehot dot.
            spc = pp.tile([P, NS], F32)
            rxy = pp.tile([P, 4], F32)   # 1/(1+exp(x)) per slot
            e4s = []
            for j in range(NS):
                e80 = pp.tile([P, C], F32, name=f"e80_{j}")
                nc.scalar.activation(out=e80[:], in_=rows[j][:, 5:85], func=AF.Exp)
                spdump2 = pp.tile([P, C], F32, name=f"spdump2_{j}")
                nc.scalar.activation(out=spdump2[:], in_=e80[:], func=AF.Ln,
                                     bias=1.0, accum_out=spc[:, j:j + 1])
                e4 = pp.tile([P, 4], F32, name=f"e4_{j}")
                nc.scalar.activation(out=e4[:], in_=rows[j][:, 0:4], func=AF.Exp)
                e4s.append(e4)
                nc.vector.tensor_scalar_add(e4[:, 0:2], e4[:, 0:2], 1.0)
                nc.vector.reciprocal(out=rxy[:, 2 * j:2 * j + 2], in_=e4[:, 0:2])

            # dxy = sigmoid - txy = (1-txy) - 1/(1+exp(x)); dwh = exp(x) - twh
            dxy = pp.tile([P, 4], F32)
            nc.vector.tensor_tensor(out=dxy[:], in0=onemt[:], in1=rxy[:], op=ALU.subtract)
            nc.vector.tensor_tensor(out=dxy[:], in0=dxy[:], in1=dxy[:], op=ALU.mult)
            dwh = pp.tile([P, 4], F32)
            for j in range(NS):
                nc.vector.tensor_tensor(out=dwh[:, 2 * j:2 * j + 2], in0=e4s[j][:, 2:4],
                                        in1=twh[:, 2 * j:2 * j + 2], op=ALU.subtract)
            nc.vector.tensor_tensor(out=dwh[:], in0=dwh[:], in1=dwh[:], op=ALU.mult)
            pxy = pp.tile([P, NS], F32)
            pwh = pp.tile([P, NS], F32)
            for j in range(NS):
                nc.vector.tensor_tensor(out=pxy[:, j:j + 1], in0=dxy[:, 2 * j:2 * j + 1],
                                        in1=dxy[:, 2 * j + 1:2 * j + 2], op=ALU.add)
                nc.vector.tensor_tensor(out=pwh[:, j:j + 1], in0=dwh[:, 2 * j:2 * j + 1],
                                        in1=dwh[:, 2 * j + 1:2 * j + 2], op=ALU.add)

            # x_cls extraction: onehot dot product per slot
            xcls = pp.tile([P, NS], F32)
            for j in range(NS):
                oh = pp.tile([P, C], F32)
                nc.vector.tensor_tensor(out=oh[:], in0=iotaf[:],
                                        in1=tt[:, 8 + j:9 + j].to_broadcast([P, C]),
                                        op=ALU.is_equal)
                nc.vector.tensor_tensor(out=oh[:], in0=oh[:], in1=rows[j][:, 5:85],
                                        op=ALU.mult)
                nc.vector.reduce_sum(out=xcls[:, j:j + 1], in_=oh[:], axis=AX.X)
            pcls = pp.tile([P, NS], F32)   # = C * per_cls
            nc.vector.tensor_tensor(out=pcls[:], in0=spc[:], in1=xcls[:], op=ALU.subtract)

            # ---- raw per-target stats -> out rows 0:100, cols 0:10
            # cols: [vf*pxy(2), vf*pwh(2), vf*pcls(2), vf(2), w*x4(2)]
            out_t = pp.tile([128, 11], F32)
            nc.vector.memset(out_t[:], 0.0)
            st = out_t[:P, :]
            nc.vector.tensor_tensor(out=st[:, 0:2], in0=pxy[:], in1=vf[:], op=ALU.mult)
            nc.vector.tensor_tensor(out=st[:, 2:4], in0=pwh[:], in1=vf[:], op=ALU.mult)
            nc.vector.tensor_tensor(out=st[:, 4:6], in0=pcls[:], in1=vf[:], op=ALU.mult)
            nc.vector.tensor_copy(out=st[:, 6:8], in_=vf[:])
            for j in range(NS):
                nc.vector.tensor_tensor(out=st[:, 8 + j:9 + j], in0=rows[j][:, 4:5],
                                        in1=wfo[:, j:j + 1], op=ALU.mult)
            nc.vector.tensor_copy(out=out_t[:, 10:11], in_=csp[:])
            nc.sync.dma_start(out=out_d.ap(), in_=out_t[:])
    if split:
        _split_multi_waits(nc)
    return nc


_NC_CACHE = None


def _get_nc():
    global _NC_CACHE
    if _NC_CACHE is None:
        _NC_CACHE = build_nc()
    return _NC_CACHE


def make_in_maps(predictions, targets):
    preds = np.ascontiguousarray(np.asarray(predictions, dtype=np.float32)).reshape(
        NCORES, ROWS, 85)
    # channel-axis shard: conf column staged contiguously per core
    conf = np.ascontiguousarray(preds[:, :, 4]).reshape(NCORES, CONF_P, CONF_F)
    # targets: [NCORES, 4, 50, 5] -> slot-packed [100, (xy slot-major, wh, cls)]
    tg = np.ascontiguousarray(np.asarray(targets, dtype=np.float32)).reshape(
        NCORES, BL, T, 5)
    # z[c, parity, t, j, f] = tg[c, 2j+parity, t, f]
    z = tg.reshape(NCORES, 2, 2, T, 5).transpose(0, 2, 3, 1, 4)  # [c,parity,t,j,f]
    cls_ = z[..., 0]                                   # [c,parity,t,j]
    xy = z[..., 1:3]                                   # [c,parity,t,j,2]
    wh = z[..., 3:5]
    tt = np.concatenate([
        xy.reshape(NCORES, P, 4),                      # {x0,y0,x1,y1}
        wh.reshape(NCORES, P, 4),                      # {w0,h0,w1,h1}
        cls_.reshape(NCORES, P, 2),                    # {cls0,cls1}
    ], axis=2)
    tt = np.ascontiguousarray(tt)
    return [{"predictions": preds[c], "conf": conf[c], "targets": tt[c]}
            for c in range(NCORES)]


def combine_partials(parts):
    """parts: list of 8 arrays [128,11] -> (total, loss_xy, loss_wh, loss_conf, loss_cls)"""
    sxy = swh = scls = nt = corr = spsum = 0.0
    for p in parts:
        a = np.asarray(p, dtype=np.float64)
        st = a[:P, 0:10]
        sxy += st[:, 0:2].sum()
        swh += st[:, 2:4].sum()
        scls += st[:, 4:6].sum()
        nt += st[:, 6:8].sum()
        corr += st[:, 8:10].sum()
        spsum += a[:, 10].sum()
    denom = max(nt, 1.0)
    loss_xy = np.float32(0.5 * sxy / denom)
    loss_wh = np.float32(0.5 * swh / denom)
    loss_cls = np.float32(scls / C / denom)
    loss_conf = np.float32((spsum - corr) / float(B * HWC))
    total = np.float32(5.0 * float(loss_xy) + 5.0 * float(loss_wh)
                       + float(loss_conf) + float(loss_cls))
    return total, loss_xy, loss_wh, loss_conf, loss_cls


def kernel(predictions, targets, H=None, W=None):
    from concourse.bass_utils import run_bass_kernel_spmd

    nc = _get_nc()
    in_maps = make_in_maps(predictions, targets)
    res = run_bass_kernel_spmd(nc, in_maps, core_ids=list(range(NCORES)))
    parts = [res.results[c]["out"] for c in range(NCORES)]
    return combine_partials(parts)


# revision 24
# speedup vs baseline: 1.2692x; 1.1819x over previous
"""Trainium2 Bass kernel for nn_MinimalLoss (YOLO-style detection loss).

Sharding strategy (data-parallel over 8 NeuronCores, 4 batches each):
  * predictions are sharded along B (each core gets its contiguous
    [4*25600, 85] slab, used only for the per-target indirect row gather);
  * the conf channel (column 4) is additionally staged as its own
    contiguous per-core [128, 800] tensor -- a channel-axis shard of
    predictions.  This turns the dominant data access (sum over all cells
    of ln(1-sigmoid(conf))) from a 4-byte-strided DMA (descriptor-rate
    bound, ~78us of SDMA busy) into one 400KB contiguous DMA (~1us).
  * targets are sharded along B and staged slot-packed/field-major as
    [100, 10] so every per-field access on device is a contiguous slice.
  * each core returns raw partial sums ([128, 11]); the final all-reduce
    of the 5 scalar loss terms happens on host in fp64.

Device math (all on-chip):
  * -ln(1-sigmoid(x)) = softplus(x): ONE activation pass with accum_out
    over the conf shard gives per-partition partial sums.
  * conf correction at an object cell: ln(1-s)-ln(s) = -x exactly, so the
    correction is just the gathered conf logit (first-occurrence weighted).
  * bce_cls per target = (sum_c softplus(x_c) - x_cls)/C exactly.
  * pred_xy = sigmoid(rows[:, 0:2]), pred_wh = exp(rows[:, 2:4]) via ACT.
  * duplicate-cell targets deduplicated with transpose/is_equal
    first-occurrence matrix per slot (2 whole batches per slot, so
    duplicates never cross slots).
"""
import numpy as np

import concourse.bass as bass
import concourse.mybir as mybir
import concourse.tile as tile
from concourse.bass import IndirectOffsetOnAxis
from concourse.masks import make_identity

F32 = mybir.dt.float32
I32 = mybir.dt.int32
AF = mybir.ActivationFunctionType
ALU = mybir.AluOpType
AX = mybir.AxisListType

B, HWC, C, T = 32, 25600, 80, 50          # full problem
H = W = 160
NCORES = 8
BL = B // NCORES                          # 4 batches per core
ROWS = BL * HWC                           # 102400 prediction rows per core
NT = BL * T                               # 200 targets per core
P = 100                                   # targets per slot (partition dim)
NS = 2                                    # slots (each = 2 whole batches)
CONF_P, CONF_F = 128, ROWS // 128         # conf shard layout [128, 800]
MAGIC = float(np.float32(2 ** 23))


def _split_multi_waits(nc):
    """Walrus codegen accepts at most ONE sync wait per instruction; hoist
    extras onto standalone EventSemaphore (wait) ops on the same engine."""
    n = 0
    for func in nc.m.functions:
        for block in func.blocks:
            out = []
            for inst in block.instructions:
                si = inst.sync_info
                if si is not None and si.on_wait and len(si.on_wait) > 1:
                    waits = list(si.on_wait)
                    for w in waits[:-1]:
                        n += 1
                        nop = mybir.InstEventSemaphore(
                            name=f"{inst.name}_sw{n}", engine=inst.engine,
                            ins=[], outs=[])
                        nop.sync_info = mybir.SyncInfo(on_wait=[w], on_update=[])
                        out.append(nop)
                    inst.sync_info = mybir.SyncInfo(on_wait=[waits[-1]],
                                                    on_update=list(si.on_update))
                out.append(inst)
            if n:
                block.instructions[:] = out
    return n


def build_nc(split=True):
    nc = bass.Bass("TRN2", target_bir_lowering=False, debug=False)
    pred_d = nc.dram_tensor("predictions", [ROWS, 85], F32, kind="ExternalInput")
    conf_d = nc.dram_tensor("conf", [CONF_P, CONF_F], F32, kind="ExternalInput")
    tgt_d = nc.dram_tensor("targets", [P, NS * 5], F32, kind="ExternalInput")
    out_d = nc.dram_tensor("out", [11, 1], F32, kind="ExternalOutput")

    with tile.TileContext(nc) as tc:
        with tc.tile_pool(name="pp", bufs=1) as pp, \
             tc.tile_pool(name="ps", bufs=1, space="PSUM") as ps:

            # ---- ACT table preload: a dummy Exp forces the exp/ln PWP set
            # to load during the prologue instead of blocking the first real
            # activation (all ACT funcs below are exp/ln = one table set).
            dummy = pp.tile([1, 1], F32)
            nc.vector.memset(dummy[:], 0.0)
            nc.scalar.activation(out=dummy[:], in_=dummy[:], func=AF.Exp)

            # ---- input DMAs, issued first on separate queues (targets head
            # the critical path, so they go out first)
            tt = pp.tile([P, NS * 5], F32)
            nc.sync.dma_start(out=tt[:], in_=tgt_d.ap())
            conf_t = pp.tile([CONF_P, CONF_F], F32)
            nc.scalar.dma_start(out=conf_t[:], in_=conf_d.ap())
            # tt cols (slot-major xy / wh, then cls):
            #   0:4  = {x0,y0,x1,y1}, 4:8 = {w0,h0,w1,h1}, 8:10 = {cls0,cls1}

            # ---- constants (DVE/gpsimd, overlap with the DMAs)
            ident_g = pp.tile([128, 128], F32)
            make_identity(nc, ident_g[:])
            ident = pp.tile([128, 128], F32)
            nc.vector.tensor_copy(out=ident[:], in_=ident_g[:])
            ones = pp.tile([128, 1], F32)
            nc.vector.memset(ones[:], 1.0)

            iotac = pp.tile([P, C], I32)
            nc.gpsimd.iota(iotac[:], pattern=[[1, C]], base=0, channel_multiplier=0)
            iotaf = pp.tile([P, C], F32)
            nc.vector.tensor_copy(out=iotaf[:], in_=iotac[:])

            iotap = pp.tile([P, 1], I32)
            nc.gpsimd.iota(iotap[:], pattern=[[1, 1]], base=0, channel_multiplier=1)
            pf = pp.tile([P, 1], F32)
            nc.vector.tensor_copy(out=pf[:], in_=iotap[:])

            iotar = pp.tile([P, P], I32)
            nc.gpsimd.iota(iotar[:], pattern=[[1, P]], base=0, channel_multiplier=0)
            iotarf = pp.tile([P, P], F32)
            nc.vector.tensor_copy(out=iotarf[:], in_=iotar[:])
            tri = pp.tile([P, P], F32)  # tri[p, f] = 1.0 iff f < p
            nc.vector.tensor_tensor(out=tri[:], in0=pf[:].to_broadcast([P, P]),
                                    in1=iotarf[:], op=ALU.is_gt)

            # negk[p, j] = -(1 + p + 100*j): unique negative dedup keys
            negi = pp.tile([P, NS], I32)
            nc.gpsimd.iota(negi[:], pattern=[[P, NS]], base=1, channel_multiplier=1)
            negk = pp.tile([P, NS], F32)
            nc.vector.tensor_copy(out=negk[:], in_=negi[:])
            nc.vector.tensor_scalar_mul(negk[:], negk[:], -1.0)

            # boff[p, j] = HWC * (2j + (p >= 50)): batch row offset
            jci = pp.tile([P, NS], I32)
            nc.gpsimd.iota(jci[:], pattern=[[1, NS]], base=0, channel_multiplier=0)
            boff = pp.tile([P, NS], F32)
            nc.vector.tensor_copy(out=boff[:], in_=jci[:])
            nc.vector.tensor_scalar_mul(boff[:], boff[:], float(2 * HWC))
            par = pp.tile([P, 1], F32)
            nc.vector.tensor_scalar(out=par[:], in0=pf[:], scalar1=float(T),
                                    scalar2=float(HWC), op0=ALU.is_ge, op1=ALU.mult)
            nc.vector.tensor_tensor(out=boff[:], in0=boff[:],
                                    in1=par[:].to_broadcast([P, NS]), op=ALU.add)

            # ---- conf term: sum softplus(conf) = sum ln(exp(conf) + 1).
            # Only exp/ln tables are used kernel-wide (one PWP table set; no
            # native softplus table on TRN2); the +1 rides the Ln bias input.
            csp = pp.tile([CONF_P, 1], F32)
            e_conf = pp.tile([CONF_P, CONF_F], F32)
            nc.scalar.activation(out=e_conf[:], in_=conf_t[:], func=AF.Exp)
            spdump = pp.tile([CONF_P, CONF_F], F32)
            nc.scalar.activation(out=spdump[:], in_=e_conf[:], func=AF.Ln,
                                 bias=1.0, accum_out=csp[:])

            # ---- per-target index chain (slot-major [P, 4] = {x0,y0,x1,y1});
            # emission order = DVE execution order, so the idx chain comes
            # first and everything gather-independent fills the gather gap.
            xw8 = pp.tile([P, 8], F32)   # {x,y}*W slot-major | {w,h}*W slot-major
            nc.vector.tensor_scalar_mul(xw8[:], tt[:, 0:8], float(W))
            xyW = xw8[:, 0:4]
            twh = xw8[:, 4:8]

            # floor in ONE fused op: round_half_even(x-0.5) == floor(x) for
            # every non-(odd-integer) x; x*W is never an exact integer here.
            # (x + (MAGIC-.5)) rounds to an integer+MAGIC, then -MAGIC.
            gxy = pp.tile([P, 4], F32)
            nc.vector.tensor_scalar(out=gxy[:], in0=xyW, scalar1=MAGIC - 0.5,
                                    scalar2=-MAGIC, op0=ALU.add, op1=ALU.add)

            # cell = gy*W + gx (strided {y0,y1} / {x0,x1} views), row index
            gcl = pp.tile([P, 4], F32)
            nc.vector.tensor_scalar(out=gcl[:], in0=gxy[:], scalar1=0.0,
                                    scalar2=float(W - 1), op0=ALU.max, op1=ALU.min)
            gv = gcl[:].rearrange("p (j c) -> p c j", c=2)     # [P, coord, slot]
            cell = pp.tile([P, NS], F32)
            cv = cell[:].rearrange("p (o j) -> p o j", o=1)    # [P, 1, slot]
            nc.vector.tensor_scalar(out=cv, in0=gv[:, 1:2, :], scalar1=float(W),
                                    scalar2=None, op0=ALU.mult)
            nc.vector.tensor_tensor(out=cv, in0=cv, in1=gv[:, 0:1, :], op=ALU.add)
            rowf = pp.tile([P, NS], F32)
            nc.vector.tensor_tensor(out=rowf[:], in0=cell[:], in1=boff[:], op=ALU.add)
            idx = pp.tile([P, NS], I32)
            nc.vector.tensor_copy(out=idx[:], in_=rowf[:])

            # ---- gather prediction rows (one indirect DMA per slot, into
            # adjacent halves of one tile so strided both-slot views work)
            rows2 = pp.tile([P, NS * 85], F32)
            for j in range(NS):
                nc.gpsimd.indirect_dma_start(
                    out=rows2[:, 85 * j:85 * j + 85], out_offset=None,
                    in_=pred_d.ap()[:, :],
                    in_offset=IndirectOffsetOnAxis(ap=idx[:, j:j + 1], axis=0))
            rows = [rows2[:, 85 * j:85 * j + 85] for j in range(NS)]

            # ---- gather-independent work (fills the gather latency):
            # validity, dedup keys, regression targets
            v4 = pp.tile([P, 4], F32)
            t4 = pp.tile([P, 4], F32)
            nc.vector.tensor_scalar(out=v4[:], in0=gxy[:], scalar1=0.0, scalar2=None,
                                    op0=ALU.is_ge)
            nc.vector.tensor_scalar(out=t4[:], in0=gxy[:], scalar1=float(W), scalar2=None,
                                    op0=ALU.is_lt)
            nc.vector.tensor_tensor(out=v4[:], in0=v4[:], in1=t4[:], op=ALU.mult)
            vf = pp.tile([P, NS], F32)
            v4v = v4[:].rearrange("p (j c) -> p j c", c=2)
            nc.vector.tensor_tensor(out=vf[:].rearrange("p (j o) -> p j o", o=1),
                                    in0=v4v[:, :, 0:1], in1=v4v[:, :, 1:2], op=ALU.mult)

            # dedup key: valid -> rowf ; invalid -> unique negative
            key = pp.tile([P, NS], F32)
            nc.vector.tensor_tensor(out=key[:], in0=rowf[:], in1=negk[:], op=ALU.subtract)
            nc.vector.tensor_tensor(out=key[:], in0=key[:], in1=vf[:], op=ALU.mult)
            nc.vector.tensor_tensor(out=key[:], in0=key[:], in1=negk[:], op=ALU.add)

            # txy and 1-txy (dxy = sigmoid - txy = (1-txy) - 1/(1+exp(x)))
            txy = pp.tile([P, 4], F32)
            nc.vector.tensor_tensor(out=txy[:], in0=xyW, in1=gxy[:], op=ALU.subtract)
            onemt = pp.tile([P, 4], F32)
            nc.vector.tensor_scalar(out=onemt[:], in0=txy[:], scalar1=-1.0, scalar2=1.0,
                                    op0=ALU.mult, op1=ALU.add)

            # ---- per-slot ACT passes (exp/ln only):
            #   sum_c softplus(cls logits) via ln(1+exp(x)) with accum_out;
            #   e4 = exp(xywh logits): wh uses it directly, sigmoid = 1-1/(1+e).
            # ---- dedup (gather-independent): first-occurrence weight per slot
            dup = pp.tile([P, NS], F32)
            for j in range(NS):
                keyT_ps = ps.tile([P, P], F32, space="PSUM", tag=f"keyT{j}")
                nc.tensor.transpose(out=keyT_ps[:], in_=key[:, j:j + 1].to_broadcast([P, P]),
                                    identity=ident[:P, :P])
                keyT_sb = pp.tile([P, P], F32)
                nc.vector.tensor_copy(out=keyT_sb[:], in_=keyT_ps[:])
                nc.vector.tensor_tensor(out=keyT_sb[:], in0=key[:, j:j + 1].to_broadcast([P, P]),
                                        in1=keyT_sb[:], op=ALU.is_equal)
                nc.vector.tensor_tensor(out=keyT_sb[:], in0=keyT_sb[:], in1=tri[:], op=ALU.mult)
                nc.vector.reduce_max(out=dup[:, j:j + 1], in_=keyT_sb[:], axis=AX.X)
            wfo = pp.tile([P, NS], F32)
            nc.vector.tensor_scalar(out=wfo[:], in0=dup[:], scalar1=-1.0, scalar2=1.0,
                                    op0=ALU.mult, op1=ALU.add)
            nc.vector.tensor_tensor(out=wfo[:], in0=wfo[:], in1=vf[:], op=ALU.mult)

            # ---- post-gather: ACT does softplus-sums (cls) and exp(xywh);
            # DVE finishes sigmoid via reciprocal and the onehot dot.
            spc = pp.tile([P, NS], F32)
            rxy = pp.tile([P, 4], F32)   # 1/(1+exp(x)) per slot
            e4s = []
            for j in range(NS):
                e80 = pp.tile([P, C], F32, name=f"e80_{j}")
                nc.scalar.activation(out=e80[:], in_=rows[j][:, 5:85], func=AF.Exp)
                spdump2 = pp.tile([P, C], F32, name=f"spdump2_{j}")
                nc.scalar.activation(out=spdump2[:], in_=e80[:], func=AF.Ln,
                                     bias=1.0, accum_out=spc[:, j:j + 1])
                e4 = pp.tile([P, 4], F32, name=f"e4_{j}")
                nc.scalar.activation(out=e4[:], in_=rows[j][:, 0:4], func=AF.Exp)
                e4s.append(e4)
                nc.vector.tensor_scalar_add(e4[:, 0:2], e4[:, 0:2], 1.0)
                nc.vector.reciprocal(out=rxy[:, 2 * j:2 * j + 2], in_=e4[:, 0:2])

            # dxy = sigmoid - txy = (1-txy) - 1/(1+exp(x)); dwh = exp(x) - twh
            dxy = pp.tile([P, 4], F32)
            nc.vector.tensor_tensor(out=dxy[:], in0=onemt[:], in1=rxy[:], op=ALU.subtract)
            nc.vector.tensor_tensor(out=dxy[:], in0=dxy[:], in1=dxy[:], op=ALU.mult)
            dwh = pp.tile([P, 4], F32)
            for j in range(NS):
                nc.vector.tensor_tensor(out=dwh[:, 2 * j:2 * j + 2], in0=e4s[j][:, 2:4],
                                        in1=twh[:, 2 * j:2 * j + 2], op=ALU.subtract)
            nc.vector.tensor_tensor(out=dwh[:], in0=dwh[:], in1=dwh[:], op=ALU.mult)
            pxy = pp.tile([P, NS], F32)
            pwh = pp.tile([P, NS], F32)
            dxyv = dxy[:].rearrange("p (j c) -> p j c", c=2)
            dwhv = dwh[:].rearrange("p (j c) -> p j c", c=2)
            nc.vector.tensor_tensor(out=pxy[:].rearrange("p (j o) -> p j o", o=1),
                                    in0=dxyv[:, :, 0:1], in1=dxyv[:, :, 1:2], op=ALU.add)
            nc.vector.tensor_tensor(out=pwh[:].rearrange("p (j o) -> p j o", o=1),
                                    in0=dwhv[:, :, 0:1], in1=dwhv[:, :, 1:2], op=ALU.add)

            # x_cls extraction: onehot dot product per slot
            xcls = pp.tile([P, NS], F32)
            for j in range(NS):
                oh = pp.tile([P, C], F32)
                nc.vector.tensor_tensor(out=oh[:], in0=iotaf[:],
                                        in1=tt[:, 8 + j:9 + j].to_broadcast([P, C]),
                                        op=ALU.is_equal)
                nc.vector.tensor_tensor(out=oh[:], in0=oh[:], in1=rows[j][:, 5:85],
                                        op=ALU.mult)
                nc.vector.reduce_sum(out=xcls[:, j:j + 1], in_=oh[:], axis=AX.X)
            pcls = pp.tile([P, NS], F32)   # = C * per_cls
            nc.vector.tensor_tensor(out=pcls[:], in0=spc[:], in1=xcls[:], op=ALU.subtract)

            # ---- per-target stats [P, 10], then TensorE column-sum -> [10,1]
            # cols: [vf*pxy(2), vf*pwh(2), vf*pcls(2), vf(2), w*x4(2)]
            st = pp.tile([P, 10], F32)
            nc.vector.tensor_copy(out=st[:, 6:8], in_=vf[:])
            r2v = rows2[:].rearrange("p (j c) -> p j c", c=85)
            nc.vector.tensor_tensor(out=st[:, 8:10].rearrange("p (j o) -> p j o", o=1),
                                    in0=r2v[:, :, 4:5],
                                    in1=wfo[:].rearrange("p (j o) -> p j o", o=1),
                                    op=ALU.mult)
            nc.vector.tensor_tensor(out=st[:, 2:4], in0=pwh[:], in1=vf[:], op=ALU.mult)
            nc.vector.tensor_tensor(out=st[:, 4:6], in0=pcls[:], in1=vf[:], op=ALU.mult)
            nc.vector.tensor_tensor(out=st[:, 0:2], in0=pxy[:], in1=vf[:], op=ALU.mult)

            stats_ps = ps.tile([10, 1], F32, space="PSUM")
            nc.tensor.matmul(out=stats_ps[:], lhsT=st[:], rhs=ones[:P, :],
                             start=True, stop=True)
            conf_ps = ps.tile([1, 1], F32, space="PSUM")
            nc.tensor.matmul(out=conf_ps[:], lhsT=csp[:], rhs=ones[:], start=True,
                             stop=True)
            so = pp.tile([10, 1], F32)
            nc.vector.tensor_copy(out=so[:], in_=stats_ps[:])
            co = pp.tile([1, 1], F32)
            nc.vector.tensor_copy(out=co[:], in_=conf_ps[:])
            nc.sync.dma_start(out=out_d.ap()[0:10, :], in_=so[:])
            nc.sync.dma_start(out=out_d.ap()[10:11, :], in_=co[:])
    if split:
        _split_multi_waits(nc)
    return nc


_NC_CACHE = None


def _get_nc():
    global _NC_CACHE
    if _NC_CACHE is None:
        _NC_CACHE = build_nc()
    return _NC_CACHE


def make_in_maps(predictions, targets):
    preds = np.ascontiguousarray(np.asarray(predictions, dtype=np.float32)).reshape(
        NCORES, ROWS, 85)
    # channel-axis shard: conf column staged contiguously per core
    conf = np.ascontiguousarray(preds[:, :, 4]).reshape(NCORES, CONF_P, CONF_F)
    # targets: [NCORES, 4, 50, 5] -> slot-packed [100, (xy slot-major, wh, cls)]
    tg = np.ascontiguousarray(np.asarray(targets, dtype=np.float32)).reshape(
        NCORES, BL, T, 5)
    # z[c, parity, t, j, f] = tg[c, 2j+parity, t, f]
    z = tg.reshape(NCORES, 2, 2, T, 5).transpose(0, 2, 3, 1, 4)  # [c,parity,t,j,f]
    cls_ = z[..., 0]                                   # [c,parity,t,j]
    xy = z[..., 1:3]                                   # [c,parity,t,j,2]
    wh = z[..., 3:5]
    tt = np.concatenate([
        xy.reshape(NCORES, P, 4),                      # {x0,y0,x1,y1}
        wh.reshape(NCORES, P, 4),                      # {w0,h0,w1,h1}
        cls_.reshape(NCORES, P, 2),                    # {cls0,cls1}
    ], axis=2)
    tt = np.ascontiguousarray(tt)
    return [{"predictions": preds[c], "conf": conf[c], "targets": tt[c]}
            for c in range(NCORES)]


def combine_partials(parts):
    """parts: list of 8 arrays [11,1] -> (total, loss_xy, loss_wh, loss_conf, loss_cls)"""
    sxy = swh = scls = nt = corr = spsum = 0.0
    for p in parts:
        a = np.asarray(p, dtype=np.float64).reshape(-1)
        sxy += a[0] + a[1]
        swh += a[2] + a[3]
        scls += a[4] + a[5]
        nt += a[6] + a[7]
        corr += a[8] + a[9]
        spsum += a[10]
    denom = max(nt, 1.0)
    loss_xy = np.float32(0.5 * sxy / denom)
    loss_wh = np.float32(0.5 * swh / denom)
    loss_cls = np.float32(scls / C / denom)
    loss_conf = np.float32((spsum - corr) / float(B * HWC))
    total = np.float32(5.0 * float(loss_xy) + 5.0 * float(loss_wh)
                       + float(loss_conf) + float(loss_cls))
    return total, loss_xy, loss_wh, loss_conf, loss_cls


def kernel(predictions, targets, H=None, W=None):
    from concourse.bass_utils import run_bass_kernel_spmd

    nc = _get_nc()
    in_maps = make_in_maps(predictions, targets)
    res = run_bass_kernel_spmd(nc, in_maps, core_ids=list(range(NCORES)))
    parts = [res.results[c]["out"] for c in range(NCORES)]
    return combine_partials(parts)
